# revision 1
# baseline (speedup 1.0000x reference)
"""Trainium2 Bass kernel for nn_CustomAttention (B=8, S=1024, H=1024, NH=16).

Strategy: data-parallel over batch — one batch element per NeuronCore, no
collectives. Host does layout-only prep (transposes / bf16 casts); all FLOPs
run on device.

Per-core dataflow (hsT = hidden_states[b].T in bf16, weights pre-transposed
and pre-tiled to bf16 on host):
  QT[o,l] = sum_h wqT[h,o] * hsT[h,l]   (f32 PSUM; +bq via per-partition
  KT[o,l] likewise                       tensor_scalar_add -> f32r SBUF)
  V[s,o]  = sum_h hsT[h,s] * wvT[h,o]   -> per s-tile V' [128, NH, 66] bf16:
            per head 64 cols of V plus a ones column (col 64) used as the
            moving ones-vector that produces softmax denominators.
  scoresT[s,l] per head = KT_h(stationary) . QT_h -> PSUM [128, S]
  expT = Exp(scoresT * 1/sqrt(HD)) -> bf16 SBUF (no max-subtraction: scores
         ~ N(0,1) so exp is well-conditioned)
  ctx[l,d] per (head, l-tile): stationary = expT chunk [s,128], moving =
         V'_h [s, 65] (64 V cols + ones col) accumulated over s-tiles in a
         one-bank PSUM tile [128, 130 used of 512]: cols 0:64 h0-ctx,
         64:65 h0-den, 65:129 h1-ctx, 129:130 h1-den.
  normalize: DVE reciprocal of den cols, then scalar_tensor_tensor
         out = ctx * recip + bv  (bv folded in here: sum(probs)=1).
  out[l, o-tile] tiles gather into a per-o-tile [128, ST, 128] staging tile,
  one DMA per o-tile into out[S, H].

Schedule: software-pipelined sections. Section t runs scores+exp(t),
ctx+normalize(t-2), and smears proj(t+1) k-steps plus (sections 0-1) the V'
production between the Act-paced scores so the PE never waits on the
exp->PSUM-free chain. PSUM: scores 2x2 banks, proj 1x2, ctx 2x1 = 8 banks.

Matmul cost on TRN2 is (moving free size) x cycles/row with stationary loads
free, so the transposed ctx (N=65 per 128x128 stationary) halves ctx cost vs
streaming expT as the moving operand. bf16 keeps full matmul rate and halves
DMA; end-to-end error vs the fp32 reference ~4e-3 (tolerance 2e-2).
"""
import sys

sys.path.insert(0, "/opt/trn_rl_repo")

import numpy as np
import ml_dtypes
from contextlib import ExitStack

from concourse import bacc, tile, mybir
from concourse.bass_utils import run_bass_kernel_spmd

F32 = mybir.dt.float32
F32R = mybir.dt.float32r
BF16 = mybir.dt.bfloat16
AF = mybir.ActivationFunctionType
ALU = mybir.AluOpType

P = 128
HD = 64
N_CORES = 8


def _chunks(total, size=512):
    out = []
    a = 0
    while a < total:
        out.append((a, min(a + size, total)))
        a += size
    return out


def build_program(S, H, NH, num_devices=N_CORES):
    """One SPMD program; every core runs it on its own batch element."""
    KT = H // P          # h-tiles (contraction tiles)
    NT = H // P          # o-tiles
    ST = S // P          # s-tiles / l-tiles
    HPT = P // HD        # heads per o-tile (2)
    assert NH * HD == H and HPT == 2 and S == H
    SCALE = 1.0 / float(np.sqrt(HD))

    nc = bacc.Bacc(
        "TRN2", target_bir_lowering=False, debug=False, num_devices=num_devices
    )

    hsT = nc.dram_tensor("hsT", [H, S], BF16, kind="ExternalInput")
    # wq/wk pre-tiled on host: row (t*P+p) = concat_k wqT[k*P+p, t*P:(t+1)*P]
    wqTp = nc.dram_tensor("wqTp", [NT * P, KT * P], BF16, kind="ExternalInput")
    wkTp = nc.dram_tensor("wkTp", [NT * P, KT * P], BF16, kind="ExternalInput")
    wvT = nc.dram_tensor("wvT", [H, H], BF16, kind="ExternalInput")
    bqk = nc.dram_tensor("bqk", [P, 2 * NT], F32, kind="ExternalInput")
    bv_row = nc.dram_tensor("bv_row", [1, H], F32, kind="ExternalInput")
    outD = nc.dram_tensor("out", [S, H], F32, kind="ExternalOutput")

    with tile.TileContext(nc) as tc, ExitStack() as ctx:
        consts = ctx.enter_context(tc.tile_pool(name="consts", bufs=1))
        hstp = ctx.enter_context(tc.tile_pool(name="hstp", bufs=KT))
        wstr = ctx.enter_context(tc.tile_pool(name="wstr", bufs=4))
        qkp = ctx.enter_context(tc.tile_pool(name="qkp", bufs=4))
        vvp = ctx.enter_context(tc.tile_pool(name="vvp", bufs=ST))
        expp = ctx.enter_context(tc.tile_pool(name="expp", bufs=6 * ST))
        rpl = ctx.enter_context(tc.tile_pool(name="rpl", bufs=4))
        outp = ctx.enter_context(tc.tile_pool(name="outp", bufs=3))
        big = ctx.enter_context(tc.tile_pool(name="big", bufs=2, space="PSUM"))
        prp = ctx.enter_context(tc.tile_pool(name="prp", bufs=1, space="PSUM"))
        cxp = ctx.enter_context(tc.tile_pool(name="cxp", bufs=2, space="PSUM"))

        # ---- input DMA: w(0) and first hsT tiles first (feed the PE asap) --
        wq_t = {}
        wk_t = {}

        def load_w(t):
            if t >= NT or t in wq_t:
                return
            for name, dram, store in (("wq", wqTp, wq_t), ("wk", wkTp, wk_t)):
                w = wstr.tile([P, KT, P], BF16, tag="wstr", name=f"{name}{t}")
                nc.sync.dma_start(
                    out=w[:],
                    in_=dram[t * P : (t + 1) * P, :].rearrange(
                        "p (k c) -> p k c", c=P
                    ),
                )
                store[t] = w

        load_w(0)

        # hsT tiles ride the (otherwise idle until ~10us) DVE/Act DMA queues
        # so they land in parallel with the SP-queue weight loads.
        ht = []
        for k in range(KT):
            t_ = hstp.tile([P, S], BF16, tag="ht", name=f"ht{k}")
            eng = nc.gpsimd if k % 2 == 0 else nc.scalar
            eng.dma_start(out=t_[:], in_=hsT[k * P : (k + 1) * P, :])
            ht.append(t_)

        # q/k biases: one small DMA, needed first at the proj(0) bias-add
        consts_bqk = consts.tile([P, 2 * NT], F32, tag="bqk")
        nc.sync.dma_start(out=consts_bqk[:], in_=bqk[:])

        wvp = ctx.enter_context(tc.tile_pool(name="wvp", bufs=KT))
        wv = []
        for k in range(KT):
            t_ = wvp.tile([P, H], BF16, tag="wv", name=f"wv{k}")
            nc.sync.dma_start(out=t_[:], in_=wvT[k * P : (k + 1) * P, :])
            wv.append(t_)

        load_w(1)

        # ---- bv broadcast (first needed at the first ctx normalize) ----
        bv_sb = consts.tile([1, H], F32, tag="bv")
        bvb = consts.tile([P, H], F32, tag="bvb")
        nc.sync.dma_start(out=bv_sb[:], in_=bv_row[:])
        nc.gpsimd.partition_broadcast(bvb[:], bv_sb[:])

        qt_t = {}
        kt_t = {}
        ex_t = {}  # t -> {(hh, j): exp tile}
        vv = []

        # ---- emission helpers ----
        def proj_steps(t, kpool=None):
            """Generator yielding k-step closures for Q then K of o-tile t;
            accumulates into the dedicated proj psum, drains via DVE.
            kpool: alternate pool for the K projection (preamble only)."""
            for w, bcol, store, tag, pool_, ptag in (
                (wq_t[t], t, qt_t, "qt", prp, "pr"),
                (wk_t[t], NT + t, kt_t, "kt", kpool or prp,
                 "big" if kpool is not None else "pr"),
            ):
                ps = pool_.tile([P, S], F32, tag=ptag, name=f"pps{t}{tag}")
                for k in range(KT):
                    for (a, b) in _chunks(S):
                        nc.tensor.matmul(
                            ps[:, a:b], w[:, k, :], ht[k][:, a:b],
                            start=(k == 0), stop=(k == KT - 1),
                        )
                    yield
                ot = qkp.tile([P, S], F32R, tag=tag, name=f"{tag}{t}")
                nc.vector.tensor_scalar_add(
                    ot[:], ps[:], consts_bqk[:, bcol : bcol + 1]
                )
                store[t] = ot
            while True:
                yield

        def vprime_steps():
            """Generator yielding one V' m-tile per step."""
            for m in range(ST):
                ps = big.tile([P, S], F32, tag="big", name=f"vps{m}")
                for k in range(KT):
                    lhs = ht[k][:, m * P : (m + 1) * P]
                    for (a, b) in _chunks(H):
                        nc.tensor.matmul(
                            ps[:, a:b], lhs, wv[k][:, a:b],
                            start=(k == 0), stop=(k == KT - 1),
                        )
                vt = vvp.tile([P, NH, 66], BF16, tag="vv", name=f"vv{m}")
                nc.vector.tensor_copy(
                    vt[:, :, 0:64], ps[:].rearrange("p (h d) -> p h d", d=HD)
                )
                nc.vector.memset(vt[:, :, 64:65], 1.0)
                vv.append(vt)
                yield
            while True:
                yield

        def emit_scores_exp(t, j):
            exs = ex_t.setdefault(t, {})
            for hh in range(HPT):
                r0 = hh * HD
                sc = big.tile([P, S], F32, tag="big", name=f"sc{t}_{j}_{hh}")
                for (a, b) in _chunks(S):
                    nc.tensor.matmul(
                        sc[:, a:b],
                        kt_t[t][r0 : r0 + HD, j * P : (j + 1) * P],
                        qt_t[t][r0 : r0 + HD, a:b],
                        start=True, stop=True,
                        tile_position=(r0, 0),
                    )
                e = expp.tile([P, S], BF16, tag="ex", name=f"ex{t}_{j}_{hh}")
                nc.scalar.activation(e[:], sc[:], AF.Exp, scale=SCALE)
                exs[(hh, j)] = e

        def emit_ctx(t, l, ot, norm_eng=None, cx_pool=None, cx_tag="cx"):
            exs = ex_t[t]
            pool_ = cx_pool or cxp
            cx = pool_.tile([P, 512], F32, tag=cx_tag, name=f"cx{t}_{l}")
            for hh in range(HPT):
                h = HPT * t + hh
                o = hh * 65
                for j in range(ST):
                    nc.tensor.matmul(
                        cx[:, o : o + 65],
                        exs[(hh, j)][:, l * P : (l + 1) * P],
                        vv[j][:, h, 0:65],
                        start=(j == 0), stop=(j == ST - 1),
                    )
            rc = rpl.tile([P, HPT], F32, tag="rc", name=f"rc{t}_{l}")
            dens = cx[:, 0 : 2 * 65].rearrange("p (h x) -> p h x", x=65)[:, :, 64:65]
            nc.vector.reciprocal(rc[:].rearrange("p (h x) -> p h x", x=1), dens)
            eng = norm_eng or nc.vector
            for hh in range(HPT):
                eng.scalar_tensor_tensor(
                    ot[:, l, hh * HD : (hh + 1) * HD],
                    cx[:, hh * 65 : hh * 65 + 64],
                    rc[:, hh : hh + 1],
                    bvb[:, (HPT * t + hh) * HD : (HPT * t + hh + 1) * HD],
                    ALU.mult,
                    ALU.add,
                )

        # ---- PE warm-up ----
        # The PE clock ramps to full rate only after ~3us of continuous
        # execution. Real work can't start until the first weight DMA lands
        # (~4us), so burn that wait on scratch matmuls: the ramp completes
        # before proj(0) begins. Scratch lives in the ctx psum ring (first
        # real use ~40us in) and a memset-fed SBUF tile.
        wu = consts.tile([P, 5 * P], BF16, tag="wu")
        nc.vector.memset(wu[:], 0.0)
        wups = cxp.tile([P, 512], F32, tag="cx", name="wups")
        for i in range(12):
            nc.tensor.matmul(
                wups[:], wu[:, 0:P], wu[:, P : 5 * P],
                start=(i == 0), stop=(i == 11),
            )

        # preamble: proj(0) unsmeared (Act idle anyway at start); K goes to
        # the big pool so it need not wait for Q's bias-add drain, and V'(0)
        # fills the PE while kt(0)'s bias-add drains.
        p0 = proj_steps(0, kpool=big)
        for _ in range(2 * KT + 1):
            next(p0)

        vgen = vprime_steps()
        next(vgen)
        nvp = 1  # V' tiles emitted so far

        ots = {}
        ctx_done = {}

        def ctx_unit(tc_, l, split_dma=False, norm_eng=None, cx_pool=None,
                     cx_tag="cx"):
            """Emit one ctx+normalize unit; DMA the o-tile column when all
            ST units of tc_ have been emitted (split_dma: one DMA per l)."""
            if tc_ not in ots:
                ots[tc_] = outp.tile([P, ST, P], F32, tag="ou", name=f"ou{tc_}")
                ctx_done[tc_] = 0
            emit_ctx(tc_, l, ots[tc_], norm_eng=norm_eng, cx_pool=cx_pool,
                     cx_tag=cx_tag)
            ctx_done[tc_] += 1
            if split_dma:
                if l % 2:  # DMA l-1..l as one transfer, alternating queues
                    eng = nc.scalar if l % 4 == 1 else nc.sync
                    eng.dma_start(
                        out=outD[
                            (l - 1) * P : (l + 1) * P,
                            tc_ * P : (tc_ + 1) * P,
                        ].rearrange("(l p) c -> p l c", p=P),
                        in_=ots[tc_][:, l - 1 : l + 1, :],
                    )
            elif ctx_done[tc_] == ST:
                nc.sync.dma_start(
                    out=outD[:, tc_ * P : (tc_ + 1) * P].rearrange(
                        "(l p) c -> p l c", p=P
                    ),
                    in_=ots[tc_][:],
                )

        # sections 0..NT-1: scores/exp(t), ctx(t-2), proj(t+1) smear, V' smear
        for t in range(NT):
            pgen = proj_steps(t + 1) if t + 1 < NT else None
            for j in range(ST):
                emit_scores_exp(t, j)
                # V' smear: 4 tiles in section 0 (after wv DMAs land), 4 in 1
                if nvp < ST and (t == 0 and j >= 3 or t == 1):
                    next(vgen)
                    nvp += 1
                if t >= 2:
                    ctx_unit(t - 2, j)
                if t == NT - 1 and j >= 2:
                    ctx_unit(NT - 2, j - 2)  # pull-in: no proj smear this section
                if pgen is not None:
                    next(pgen)  # 2 k-steps per j
                    next(pgen)
            if pgen is not None:
                next(pgen)  # flush the trailing bias-add
            load_w(t + 2)

        # tail: remaining ctx(NT-2) units, then ctx(NT-1) with per-l DMAs.
        # Only the drain chains remain: rotate the ctx PSUM through the
        # now-idle proj/scores rings (3 chains in flight) and alternate the
        # normalize between DVE and GPSIMD so two engines drain in parallel.
        tail_rot = [(cxp, "cx"), (prp, "pr"), (big, "big")]
        tail = [(NT - 2, ST - 2, False), (NT - 2, ST - 1, False)] + [
            (NT - 1, l, True) for l in range(ST)
        ]
        for i, (tc_, l, split) in enumerate(tail):
            pool_, tag_ = tail_rot[i % 3]
            # normalize must stay on DVE: GPSIMD cannot access PSUM on HW
            ctx_unit(tc_, l, split_dma=split, cx_pool=pool_, cx_tag=tag_)

    nc.compile()
    return nc


_CACHE = {}


def _get_program(S, H, NH, num_devices):
    key = (S, H, NH, num_devices)
    if key not in _CACHE:
        _CACHE[key] = build_program(S, H, NH, num_devices)
    return _CACHE[key]


def make_in_maps(hidden_states, Wq, bq, Wk, bk, Wv, bv):
    B, S, H = hidden_states.shape
    NT = H // P
    KT = H // P
    # wq/wk pre-tiled: row (t*P+p) holds concat over k of wT[k*P+p, t*P:(t+1)*P]
    def pack_w(W):
        wT = np.ascontiguousarray(W.T.astype(np.float32))  # [h, o]
        w4 = wT.reshape(KT, P, NT, P)                      # [k, p, t, c]
        return np.ascontiguousarray(
            w4.transpose(2, 1, 0, 3).reshape(NT * P, KT * P)
        ).astype(ml_dtypes.bfloat16)

    wqTp = pack_w(Wq)
    wkTp = pack_w(Wk)
    wvT = np.ascontiguousarray(Wv.T.astype(np.float32)).astype(ml_dtypes.bfloat16)
    bqk = np.ascontiguousarray(
        np.concatenate(
            [bq.reshape(NT, P).T, bk.reshape(NT, P).T], axis=1
        ).astype(np.float32)
    )
    bv_row = bv.astype(np.float32).reshape(1, H)
    in_maps = []
    for b in range(B):
        in_maps.append(
            {
                "hsT": np.ascontiguousarray(
                    hidden_states[b].T.astype(np.float32)
                ).astype(ml_dtypes.bfloat16),
                "wqTp": wqTp,
                "wkTp": wkTp,
                "wvT": wvT,
                "bqk": bqk,
                "bv_row": bv_row,
            }
        )
    return in_maps


def kernel(hidden_states, Wq, bq, Wk, bk, Wv, bv):
    hidden_states = np.asarray(hidden_states, dtype=np.float32)
    Wq = np.asarray(Wq, dtype=np.float32)
    bq = np.asarray(bq, dtype=np.float32)
    Wk = np.asarray(Wk, dtype=np.float32)
    bk = np.asarray(bk, dtype=np.float32)
    Wv = np.asarray(Wv, dtype=np.float32)
    bv = np.asarray(bv, dtype=np.float32)

    B, S, H = hidden_states.shape
    NH = H // HD
    assert B == N_CORES, "one batch element per core"

    nc = _get_program(S, H, NH, N_CORES)
    in_maps = make_in_maps(hidden_states, Wq, bq, Wk, bk, Wv, bv)
    res = run_bass_kernel_spmd(nc, in_maps, core_ids=list(range(N_CORES)))
    out = np.empty((B, S, H), np.float32)
    for b in range(B):
        out[b] = res.results[b]["out"]
    return out


if __name__ == "__main__":
    build_program(1024, 1024, 16)
    print("build ok")



# revision 10
# speedup vs baseline: 1.0360x; 1.0360x over previous
"""Trainium2 Bass kernel for nn_CustomAttention (B=8, S=1024, H=1024, NH=16).

Strategy: data-parallel over batch — one batch element per NeuronCore, no
collectives. Host does layout-only prep (transposes / fp8+bf16 casts); all
FLOPs run on device.

v3: the Q/K/V projections run as fp8e4m3 DoubleRow matmuls (2 output
cols/cycle) with a 3-term RESIDUAL decomposition that keeps full accuracy:
  A  = fp8(32*W.T)              (weights in e4m3's sweet spot)
  B  = fp8(1024*(W.T - A/32))   (weight quantization residual)
  h8 = fp8(hs.T), d8 = fp8(32*(hs.T - h8))   (activation + residual)
  psA = h8@A + d8@A   (scale 32*q)     psB = h8@B   (scale 1024*q-residual)
  qt  = psA + psB/32 + 32*bq   (DVE: scalar_tensor_tensor + tensor_scalar_add)
Dropped terms are O(eps^2) ~ 0.1%, so logit noise stays ~0.002 (budget 0.015
— plain fp8's 0.04 logit noise fails the 2e-2 gate on concentrated softmax
rows). Projections cost 12 DR c-steps per 512-col chunk = 0.75x of bf16.
Scores / exp / ctx stay bf16 (exact-enough): fp8 anywhere in the softmax
value path was measured over-tolerance.

Scale bookkeeping: qt/kt = 32*(q+bq) bf16 -> scores psum = 1024*logits*8;
exp scale = (1/8)/1024; V' = 32*V bf16 with ones-col 32.0 so the DVE
reciprocal of den = 1/(32*Z) cancels everything in the normalize.

Per-core dataflow (as v1 otherwise):
  scoresT[s,l] per head = kt_h(stationary) . qt_h -> PSUM [128, S]
  expT = Exp(scores * scale) -> bf16 SBUF
  ctx[l,d] per (head, l): stationary expT chunk, moving V'[s, 65] (64 V cols
    + ones col) accumulated over s-tiles in one PSUM bank.
  normalize: DVE reciprocal + scalar_tensor_tensor (folds bv).
Schedule: software-pipelined sections; section t runs scores+exp(t), a ctx
work-queue (quota-paced so PE never outruns Act), proj(t+1) chunk-steps, and
V' chunk-steps in sections 0-2. PE is the critical engine (~144us busy);
Act's exp stream is ~133us; all steady-state DMAs stay off the Act queue.
PSUM: scores 2x2 banks, projA 1, projB 1, ctx 2x1 = 8 banks.
"""
import sys

sys.path.insert(0, "/opt/trn_rl_repo")

import numpy as np
import ml_dtypes
from collections import deque
from contextlib import ExitStack

from concourse import bacc, tile, mybir
from concourse.bass_utils import run_bass_kernel_spmd

F32 = mybir.dt.float32
BF16 = mybir.dt.bfloat16
FP8 = mybir.dt.float8e4
AF = mybir.ActivationFunctionType
ALU = mybir.AluOpType
DR = mybir.MatmulPerfMode.DoubleRow

P = 128
HD = 64
N_CORES = 8
WSCALE = 32.0
RESCALE = 1024.0  # weight-residual scale (WSCALE * WSCALE)
RINV = 1.0 / WSCALE

FP8NP = ml_dtypes.float8_e4m3


def _chunks(total, size=512):
    out = []
    a = 0
    while a < total:
        out.append((a, min(a + size, total)))
        a += size
    return out


def build_program(S, H, NH, num_devices=N_CORES):
    """One SPMD program; every core runs it on its own batch element."""
    KT = H // P          # h-tiles (contraction tiles)
    KP = KT // 2         # DoubleRow contraction pairs
    NT = H // P          # o-tiles
    ST = S // P          # s-tiles / l-tiles
    HPT = P // HD        # heads per o-tile (2)
    assert NH * HD == H and HPT == 2 and S == H
    SCALE = 1.0 / float(np.sqrt(HD))
    EXP_SCALE = SCALE / (WSCALE * WSCALE)

    nc = bacc.Bacc(
        "TRN2", target_bir_lowering=False, debug=False, num_devices=num_devices
    )

    # hsT pairs: row (c*P+p) = concat_i X[(2c+i)*P+p, :]
    h8p = nc.dram_tensor("h8p", [KP * P, 2 * S], FP8, kind="ExternalInput")
    d8p = nc.dram_tensor("d8p", [KP * P, 2 * S], FP8, kind="ExternalInput")
    # wq/wk DoubleRow-packed: row (t*P+p), col (c*2P + i*P + m)
    wqA = nc.dram_tensor("wqA", [NT * P, KP * 2 * P], FP8, kind="ExternalInput")
    wqB = nc.dram_tensor("wqB", [NT * P, KP * 2 * P], FP8, kind="ExternalInput")
    wkA = nc.dram_tensor("wkA", [NT * P, KP * 2 * P], FP8, kind="ExternalInput")
    wkB = nc.dram_tensor("wkB", [NT * P, KP * 2 * P], FP8, kind="ExternalInput")
    # wv pairs (moving operand layout)
    wvA = nc.dram_tensor("wvA", [KP * P, 2 * H], FP8, kind="ExternalInput")
    wvB = nc.dram_tensor("wvB", [KP * P, 2 * H], FP8, kind="ExternalInput")
    bqk = nc.dram_tensor("bqk", [P, 2 * NT], F32, kind="ExternalInput")
    bv_row = nc.dram_tensor("bv_row", [1, H], F32, kind="ExternalInput")
    outD = nc.dram_tensor("out", [S, H], F32, kind="ExternalOutput")

    with tile.TileContext(nc) as tc, ExitStack() as ctx:
        consts = ctx.enter_context(tc.tile_pool(name="consts", bufs=1))
        hstp = ctx.enter_context(tc.tile_pool(name="hstp", bufs=2 * KP))
        wstr = ctx.enter_context(tc.tile_pool(name="wstr", bufs=8))
        qkp = ctx.enter_context(tc.tile_pool(name="qkp", bufs=4))
        tmpp = ctx.enter_context(tc.tile_pool(name="tmpp", bufs=4))
        vvp = ctx.enter_context(tc.tile_pool(name="vvp", bufs=ST))
        expp = ctx.enter_context(tc.tile_pool(name="expp", bufs=6 * ST))
        rpl = ctx.enter_context(tc.tile_pool(name="rpl", bufs=4))
        outp = ctx.enter_context(tc.tile_pool(name="outp", bufs=3))
        big = ctx.enter_context(tc.tile_pool(name="big", bufs=2, space="PSUM"))
        prA = ctx.enter_context(tc.tile_pool(name="prA", bufs=1, space="PSUM"))
        prB = ctx.enter_context(tc.tile_pool(name="prB", bufs=1, space="PSUM"))
        cxp = ctx.enter_context(tc.tile_pool(name="cxp", bufs=2, space="PSUM"))

        # ---- input DMA: w(0) and hsT tiles first (feed the PE asap) ------
        w_t = {}  # (mat, t) -> (A tile, B tile)

        def load_w(t):
            if t >= NT or ("q", t) in w_t:
                return
            for mat, dA, dB in (("q", wqA, wqB), ("k", wkA, wkB)):
                tiles = []
                for nm, dram in ((f"w{mat}A{t}", dA), (f"w{mat}B{t}", dB)):
                    w = wstr.tile([P, KP, 2, P], FP8, tag="wstr", name=nm)
                    nc.sync.dma_start(
                        out=w[:],
                        in_=dram[t * P : (t + 1) * P, :].rearrange(
                            "p (c i m) -> p c i m", c=KP, i=2
                        ),
                    )
                    tiles.append(w)
                w_t[(mat, t)] = tiles

        load_w(0)

        # h8/d8 pair tiles ride the gpsimd/Act DMA queues so they land in
        # parallel with the SP-queue weight loads. Act is fine ONLY here:
        # its first exp is ~7us out; steady-state DMAs stay off Act.
        h8 = []
        d8 = []
        for c in range(KP):
            t_ = hstp.tile([P, 2, S], FP8, tag="ht", name=f"h8_{c}")
            nc.gpsimd.dma_start(
                out=t_[:],
                in_=h8p[c * P : (c + 1) * P, :].rearrange("p (i l) -> p i l", i=2),
            )
            h8.append(t_)
        for c in range(KP):
            t_ = hstp.tile([P, 2, S], FP8, tag="ht", name=f"d8_{c}")
            nc.scalar.dma_start(
                out=t_[:],
                in_=d8p[c * P : (c + 1) * P, :].rearrange("p (i l) -> p i l", i=2),
            )
            d8.append(t_)

        # q/k biases: one small DMA, needed first at the proj(0) drain
        consts_bqk = consts.tile([P, 2 * NT], F32, tag="bqk")
        nc.sync.dma_start(out=consts_bqk[:], in_=bqk[:])

        # wv moving tiles: needed from section 0 j>=2; split across queues
        wvpool = ctx.enter_context(tc.tile_pool(name="wvpool", bufs=2 * KP))
        wva = []
        wvb = []
        for c in range(KP):
            t_ = wvpool.tile([P, 2, H], FP8, tag="wv", name=f"wvA{c}")
            nc.gpsimd.dma_start(
                out=t_[:],
                in_=wvA[c * P : (c + 1) * P, :].rearrange("p (i o) -> p i o", i=2),
            )
            wva.append(t_)
        for c in range(KP):
            t_ = wvpool.tile([P, 2, H], FP8, tag="wv", name=f"wvB{c}")
            nc.scalar.dma_start(
                out=t_[:],
                in_=wvB[c * P : (c + 1) * P, :].rearrange("p (i o) -> p i o", i=2),
            )
            wvb.append(t_)

        load_w(1)

        # ---- bv broadcast (first needed at the first ctx normalize) ----
        bv_sb = consts.tile([1, H], F32, tag="bv")
        bvb = consts.tile([P, H], F32, tag="bvb")
        nc.sync.dma_start(out=bv_sb[:], in_=bv_row[:])
        nc.gpsimd.partition_broadcast(bvb[:], bv_sb[:])

        qt_t = {}
        kt_t = {}
        ex_t = {}  # t -> {(hh, j): exp tile}
        vv = []

        # ---- emission helpers ----
        def proj_chunk(wa, wb, a, b, ot, bcol, psA_ap=None, psB_ap=None):
            """12 DR matmuls + 2-op DVE drain for cols a:b of one matrix."""
            if psA_ap is None:
                psA_ap = prA.tile([P, 512], F32, tag="prA", name="pA")[:]
                psB_ap = prB.tile([P, 512], F32, tag="prB", name="pB")[:]
            # psA = h8@A (scale 32*q); psB = d8@A + h8@B (both scale 1024):
            # d8 and B each carry a 32x residual boost, so they share psB.
            for c in range(KP):
                nc.tensor.matmul(
                    psA_ap, wa[:, c, :, :], h8[c][:, :, a:b],
                    start=(c == 0), stop=(c == KP - 1), perf_mode=DR,
                )
            for c in range(KP):
                nc.tensor.matmul(
                    psB_ap, wa[:, c, :, :], d8[c][:, :, a:b],
                    start=(c == 0), stop=False, perf_mode=DR,
                )
            for c in range(KP):
                nc.tensor.matmul(
                    psB_ap, wb[:, c, :, :], h8[c][:, :, a:b],
                    start=False, stop=(c == KP - 1), perf_mode=DR,
                )
            # ISA s2s2d2: two PSUM tensor srcs are illegal, so combine in two
            # one-PSUM ops: tmp = psB/32 + 32*bias, then qt = psA + tmp.
            tmp = tmpp.tile([P, 512], BF16, tag="tmp", name="tmp")
            nc.vector.tensor_scalar(
                tmp[:], psB_ap, RINV, consts_bqk[:, bcol : bcol + 1],
                ALU.mult, ALU.add,
            )
            nc.vector.tensor_tensor(ot[:, a:b], psA_ap, tmp[:], ALU.add)

        def proj_steps(t, kbig=False):
            """Generator yielding per (matrix, 512-col chunk): Q then K of
            o-tile t. kbig: K's psums live in one big-pool tile (preamble
            only, so K need not wait on Q's psum drain)."""
            for mat, bcol, store, tag in (("q", t, qt_t, "qt"),
                                          ("k", NT + t, kt_t, "kt")):
                wa, wb = w_t[(mat, t)]
                ot = qkp.tile([P, S], BF16, tag=tag, name=f"{tag}{t}")
                store[t] = ot
                for (a, b) in _chunks(S):
                    if kbig and mat == "k":
                        ps = big.tile([P, S], F32, tag="big", name=f"kps{a}")
                        proj_chunk(wa, wb, a, b, ot, bcol,
                                   ps[:, 0:512], ps[:, 512:1024])
                    else:
                        proj_chunk(wa, wb, a, b, ot, bcol)
                    yield
            while True:
                yield

        def vprime_steps():
            """Generator yielding one V' 512-col chunk per step (2 chunks
            per s-tile m). V' = 32*V in bf16; col 64 = 32.0 (the moving
            ones-column that produces softmax denominators)."""
            for m in range(ST):
                vt = vvp.tile([P, NH, 66], BF16, tag="vv", name=f"vv{m}")
                for ci, (a, b) in enumerate(_chunks(H)):
                    psA = prA.tile([P, 512], F32, tag="prA", name=f"vA{m}_{ci}")
                    psB = prB.tile([P, 512], F32, tag="prB", name=f"vB{m}_{ci}")
                    for c in range(KP):
                        nc.tensor.matmul(
                            psA[:], h8[c][:, :, m * P : (m + 1) * P],
                            wva[c][:, :, a:b],
                            start=(c == 0), stop=(c == KP - 1), perf_mode=DR,
                        )
                    for c in range(KP):
                        nc.tensor.matmul(
                            psB[:], d8[c][:, :, m * P : (m + 1) * P],
                            wva[c][:, :, a:b],
                            start=(c == 0), stop=False, perf_mode=DR,
                        )
                    for c in range(KP):
                        nc.tensor.matmul(
                            psB[:], h8[c][:, :, m * P : (m + 1) * P],
                            wvb[c][:, :, a:b],
                            start=False, stop=(c == KP - 1), perf_mode=DR,
                        )
                    h0 = a // HD
                    tmpv = tmpp.tile([P, 512], BF16, tag="tmp", name="tmpv")
                    nc.vector.tensor_scalar_mul(tmpv[:], psB[:], RINV)
                    nc.vector.tensor_tensor(
                        vt[:, h0 : h0 + 512 // HD, 0:64],
                        psA[:].rearrange("p (h d) -> p h d", d=HD),
                        tmpv[:].rearrange("p (h d) -> p h d", d=HD),
                        ALU.add,
                    )
                    if ci == 1:
                        nc.vector.memset(vt[:, :, 64:65], WSCALE)
                        vv.append(vt)
                    yield
            while True:
                yield

        def emit_scores_exp(t, j):
            exs = ex_t.setdefault(t, {})
            for hh in range(HPT):
                r0 = hh * HD
                sc = big.tile([P, S], F32, tag="big", name=f"sc{t}_{j}_{hh}")
                for (a, b) in _chunks(S):
                    nc.tensor.matmul(
                        sc[:, a:b],
                        kt_t[t][r0 : r0 + HD, j * P : (j + 1) * P],
                        qt_t[t][r0 : r0 + HD, a:b],
                        start=True, stop=True,
                        tile_position=(r0, 0),
                    )
                e = expp.tile([P, S], BF16, tag="ex", name=f"ex{t}_{j}_{hh}")
                nc.scalar.activation(e[:], sc[:], AF.Exp, scale=EXP_SCALE)
                exs[(hh, j)] = e

        def emit_ctx(t, l, ot, cx_pool=None, cx_tag="cx"):
            exs = ex_t[t]
            pool_ = cx_pool or cxp
            cx = pool_.tile([P, 512], F32, tag=cx_tag, name=f"cx{t}_{l}")
            for hh in range(HPT):
                h = HPT * t + hh
                o = hh * 65
                for j in range(ST):
                    nc.tensor.matmul(
                        cx[:, o : o + 65],
                        exs[(hh, j)][:, l * P : (l + 1) * P],
                        vv[j][:, h, 0:65],
                        start=(j == 0), stop=(j == ST - 1),
                    )
            rc = rpl.tile([P, HPT], F32, tag="rc", name=f"rc{t}_{l}")
            dens = cx[:, 0 : 2 * 65].rearrange("p (h x) -> p h x", x=65)[:, :, 64:65]
            nc.vector.reciprocal(rc[:].rearrange("p (h x) -> p h x", x=1), dens)
            for hh in range(HPT):
                nc.vector.scalar_tensor_tensor(
                    ot[:, l, hh * HD : (hh + 1) * HD],
                    cx[:, hh * 65 : hh * 65 + 64],
                    rc[:, hh : hh + 1],
                    bvb[:, (HPT * t + hh) * HD : (HPT * t + hh + 1) * HD],
                    ALU.mult,
                    ALU.add,
                )

        # ---- PE warm-up ----
        # The PE clock ramps to full rate only after ~3us of continuous
        # execution; real work can't start until the first weight/hs DMAs
        # land, so burn that wait on scratch matmuls.
        wu = consts.tile([P, 5 * P], BF16, tag="wu")
        nc.vector.memset(wu[:], 0.0)
        wups = cxp.tile([P, 512], F32, tag="cx", name="wups")
        for i in range(8):
            nc.tensor.matmul(
                wups[:, 0:256], wu[:, 0:P], wu[:, P : P + 256],
                start=(i == 0), stop=(i == 7),
            )

        # preamble: proj(0) unsmeared (Act idle anyway at start); K's psums
        # go to the big pool so they need not wait on Q's chunk drains.
        p0 = proj_steps(0, kbig=True)
        for _ in range(4):
            next(p0)

        vgen = vprime_steps()
        nvp = 0  # V' chunk-steps emitted so far

        ots = {}
        ctx_done = {}

        def ctx_unit(tc_, l, split_dma=False, cx_pool=None, cx_tag="cx"):
            """Emit one ctx+normalize unit; DMA the o-tile column when all
            ST units of tc_ have been emitted (split_dma: one DMA per l)."""
            if tc_ not in ots:
                ots[tc_] = outp.tile([P, ST, P], F32, tag="ou", name=f"ou{tc_}")
                ctx_done[tc_] = 0
            emit_ctx(tc_, l, ots[tc_], cx_pool=cx_pool, cx_tag=cx_tag)
            ctx_done[tc_] += 1
            if split_dma:
                if l % 2:  # DMA l-1..l as one transfer, alternating queues
                    eng = nc.gpsimd if l % 4 == 1 else nc.sync
                    eng.dma_start(
                        out=outD[
                            (l - 1) * P : (l + 1) * P,
                            tc_ * P : (tc_ + 1) * P,
                        ].rearrange("(l p) c -> p l c", p=P),
                        in_=ots[tc_][:, l - 1 : l + 1, :],
                    )
            elif ctx_done[tc_] == ST:
                nc.gpsimd.dma_start(
                    out=outD[:, tc_ * P : (tc_ + 1) * P].rearrange(
                        "(l p) c -> p l c", p=P
                    ),
                    in_=ots[tc_][:],
                )

        # ---- sections 0..NT-1 ----
        # scores/exp(t) pace Act; proj(t+1) smears 4 chunk-steps at odd j;
        # V' smears its 16 chunk-steps over sections 0-2; ctx units drain
        # from a quota-paced queue (tile t-1 units only at j>=3: its last
        # exps land ~2us into section t).
        cqueue = deque()

        def vslots(t):
            return {0: range(2, ST), 1: range(0, 6), 2: range(0, 4)}.get(t, ())

        def cquota(t, j):
            if t == 2:
                return 1 if j >= 4 else 0
            if 3 <= t <= 6:
                return 1 + (1 if j >= 6 else 0)
            if t == 7:
                return 2
            return 0

        for t in range(NT):
            pgen = proj_steps(t + 1) if t + 1 < NT else None
            if t >= 1:
                cqueue.extend((t - 1, l) for l in range(ST))
            for j in range(ST):
                emit_scores_exp(t, j)
                if j in vslots(t):
                    next(vgen)
                    nvp += 1
                npop = cquota(t, j)
                while npop > 0 and cqueue and (
                    cqueue[0][0] <= t - 2 or j >= 3
                ):
                    tc_, l = cqueue.popleft()
                    ctx_unit(tc_, l)
                    npop -= 1
                if pgen is not None and j % 2 == 1:
                    next(pgen)
            load_w(t + 2)
        assert nvp == 2 * ST

        # tail: remaining ctx units (tile 7, plus any stragglers), rotating
        # the ctx PSUM through the now-idle proj/scores rings.
        cqueue.extend((NT - 1, l) for l in range(ST))
        tail_rot = [(cxp, "cx"), (prA, "prA"), (big, "big")]
        i = 0
        while cqueue:
            tc_, l = cqueue.popleft()
            pool_, tag_ = tail_rot[i % 3]
            i += 1
            ctx_unit(tc_, l, split_dma=(tc_ == NT - 1), cx_pool=pool_,
                     cx_tag=tag_)

    nc.compile()
    return nc


_CACHE = {}


def _get_program(S, H, NH, num_devices):
    key = (S, H, NH, num_devices)
    if key not in _CACHE:
        _CACHE[key] = build_program(S, H, NH, num_devices)
    return _CACHE[key]


def make_in_maps(hidden_states, Wq, bq, Wk, bk, Wv, bv):
    B, S, H = hidden_states.shape
    NT = H // P
    KT = H // P
    KP = KT // 2

    def pack_pair_rows(X):
        # [KT*P, W] -> [KP*P, 2W]: row c*P+p = concat_i X[(2c+i)*P+p, :]
        r = np.ascontiguousarray(X).reshape(KP, 2, P, -1)
        return np.ascontiguousarray(r.transpose(0, 2, 1, 3).reshape(KP * P, -1))

    def pack_w_dr(wT):
        # DoubleRow stationary pack: row (t*P+p), col (c*2P+i*P+m)
        #   = wT[(2c+i)*P+p, t*P+m]
        w5 = np.ascontiguousarray(wT).reshape(KP, 2, P, NT, P)
        return np.ascontiguousarray(
            w5.transpose(3, 2, 0, 1, 4).reshape(NT * P, KP * 2 * P)
        )

    def residual_pair(wT):
        A = (WSCALE * wT).astype(FP8NP)
        Bm = (RESCALE * (wT - A.astype(np.float32) / WSCALE)).astype(FP8NP)
        return A, Bm

    qA, qB = residual_pair(np.ascontiguousarray(Wq.T.astype(np.float32)))
    kA, kB = residual_pair(np.ascontiguousarray(Wk.T.astype(np.float32)))
    vA, vB = residual_pair(np.ascontiguousarray(Wv.T.astype(np.float32)))

    wqA_ = pack_w_dr(qA)
    wqB_ = pack_w_dr(qB)
    wkA_ = pack_w_dr(kA)
    wkB_ = pack_w_dr(kB)
    wvA_ = pack_pair_rows(vA)
    wvB_ = pack_pair_rows(vB)

    bqk = np.ascontiguousarray(
        np.concatenate(
            [bq.reshape(NT, P).T, bk.reshape(NT, P).T], axis=1
        ).astype(np.float32)
    ) * WSCALE
    bv_row = bv.astype(np.float32).reshape(1, H)

    in_maps = []
    for b in range(B):
        hsT = np.ascontiguousarray(hidden_states[b].T.astype(np.float32))
        h8 = hsT.astype(FP8NP)
        d8 = (WSCALE * (hsT - h8.astype(np.float32))).astype(FP8NP)
        in_maps.append(
            {
                "h8p": pack_pair_rows(h8),
                "d8p": pack_pair_rows(d8),
                "wqA": wqA_,
                "wqB": wqB_,
                "wkA": wkA_,
                "wkB": wkB_,
                "wvA": wvA_,
                "wvB": wvB_,
                "bqk": bqk,
                "bv_row": bv_row,
            }
        )
    return in_maps


def kernel(hidden_states, Wq, bq, Wk, bk, Wv, bv):
    hidden_states = np.asarray(hidden_states, dtype=np.float32)
    Wq = np.asarray(Wq, dtype=np.float32)
    bq = np.asarray(bq, dtype=np.float32)
    Wk = np.asarray(Wk, dtype=np.float32)
    bk = np.asarray(bk, dtype=np.float32)
    Wv = np.asarray(Wv, dtype=np.float32)
    bv = np.asarray(bv, dtype=np.float32)

    B, S, H = hidden_states.shape
    NH = H // HD
    assert B == N_CORES, "one batch element per core"

    nc = _get_program(S, H, NH, N_CORES)
    in_maps = make_in_maps(hidden_states, Wq, bq, Wk, bk, Wv, bv)
    res = run_bass_kernel_spmd(nc, in_maps, core_ids=list(range(N_CORES)))
    out = np.empty((B, S, H), np.float32)
    for b in range(B):
        out[b] = res.results[b]["out"]
    return out


if __name__ == "__main__":
    build_program(1024, 1024, 16)
    print("build ok")
